# revision 9
# baseline (speedup 1.0000x reference)
"""Trainium2 8-core Bass kernel: out = sigmoid(encoder_outputs @ hidden),
encoder_outputs [32768, 1024] f32, hidden [1024] f32 -> [1, 1, 32768] f32.

Sharding: encoder_outputs splits along seq_len into 8 slices of [4096, 1024]
(one per NeuronCore); hidden is replicated; each core produces its 4096
sigmoid scores and the host concatenates. No collectives needed.

Per-core kernel (raw bacc, hand-placed semaphores; no Tile machinery):
  - partition p owns rows [32p, 32p+32) of the slice; scores map to the
    output vector with per-partition contiguous stores
  - rows 0-1 load f32 via HWDGE (SP) before the SWDGE ring finishes
    initializing; rows 2-26 stream as full-width SWDGE cast-DMAs
    (f32 DRAM -> bf16 SBUF)
  - SDMA engine 15 (partitions 92-95/124-127 by the port swizzle) is the
    known-slow engine under SWDGE descriptor-ring contention, so those 8
    partitions only receive rows 0-26: their rows 27-31 are instead loaded
    into 40 "donor" partitions (0-19, 64-83) via two 3D-AP cast-DMAs,
    reduced there, and scatter-stored straight to the right DRAM words
    mid-kernel. Rows 27-31 for the remaining 120 partitions stream as
    partition-subset loads [0:92]/[96:124] that skip engine 15 entirely.
  - each row is multiplied and reduced in ONE DVE op
    (tensor_tensor_reduce: out = E_row * h, accum_out = row-sum in f32),
    so ScalarE only runs sigmoids and the post-last-byte chain is just
    TTR -> sigmoid -> store
  - a warm Sigmoid on the const-zero AP pulls the single ACT funcset load
    off the tail; branch hints arm each engine's end-block branch
Memory-bound at the per-engine SDMA line rate (~25.7 GB/s f32-read per
engine with cast); bf16 multiply keeps rel err ~5e-3 (gate 2e-2).
"""
import numpy as np
from concourse.bass_utils import run_bass_kernel_spmd


import concourse.bass as bass
from concourse import bacc, mybir


class _HintedBlock(bass.BassBlock):
    """no_gpsimd_drain block whose end-bb branches carry prefetch hints."""

    def __init__(self, bass_, name):
        super().__init__(bass_, name, no_gpsimd_drain=True)
        self.hint_locs = {}

    def __exit__(self, exc_type, exc_val, exc_tb):
        if exc_type is not None:
            return
        for engine, last_body in self.last_body.items():
            with self.bass.body(last_body, parent=self.bass.cur_bb,
                                allow_existing_parent=True):
                br = engine.br(self.end_bb)
                loc = self.hint_locs.get(engine)
                if loc is not None:
                    br.branch_hint(loc)
        self.bass.switch_bb(self.end_bb)
        gpsimd_type = self.bass.gpsimd.engine
        for eng_type, eng in self.bass.engines.items():
            if eng_type == gpsimd_type:
                continue
            d = mybir.InstDrain(
                name=self.bass.get_next_instruction_name(),
                ins=[], outs=[], bass_is_fusable=False)
            d.engine = eng_type
            eng.add_instruction(d)
        self.bass.all_engine_barrier(sem_only=True)

N_CORES = 8
SEQ = 32768
D = 1024
ROWS = SEQ // N_CORES          # 4096
RPP = ROWS // 128              # 32
F32 = mybir.dt.float32
BF16 = mybir.dt.bfloat16

HEAD_ROWS = 2                  # rows 0-1, f32 via HWDGE at kernel start
MAIN_SIZES = [4, 4, 4, 4, 4, 5]  # rows 2..26, full 128-partition SWDGE
TAIL_SIZES = [2, 1, 1, 1]        # rows 27..31, partitions [0:92]+[96:124]
# slow-engine partitions (SDMA engine 15): 92-95 and 124-127.
# their rows 27..31 are loaded by donor partitions [0:20] and [64:84].
N_MAIN_ROWS = sum(MAIN_SIZES)    # 25
N_TAIL_ROWS = sum(TAIL_SIZES)    # 5
TAIL0 = HEAD_ROWS + N_MAIN_ROWS  # 27
assert TAIL0 + N_TAIL_ROWS == RPP


def build():
    nc = bacc.Bacc("TRN2", target_bir_lowering=False, debug=False,
                   num_devices=N_CORES)
    h_dram = nc.dram_tensor("hidden", [D], F32, kind="ExternalInput")
    e_dram = nc.dram_tensor("encoder_outputs", [ROWS, D], F32,
                            kind="ExternalInput")
    o_dram = nc.dram_tensor("out", [ROWS], F32, kind="ExternalOutput")
    ev3 = e_dram.ap().rearrange("(p r) d -> p r d", p=128)   # [128, 32, D]
    o_rear = o_dram.ap().rearrange("(p r) -> p r", p=128)    # [128, 32]

    eallf = nc.alloc_sbuf_tensor("eallf", [128, HEAD_ROWS * D], F32)
    eall = nc.alloc_sbuf_tensor("eall", [128, (RPP - HEAD_ROWS) * D], BF16)
    extra = nc.alloc_sbuf_tensor("extra", [128, D], BF16)  # donor rows
    htf = nc.alloc_sbuf_tensor("htf", [128, D], F32)
    ht = nc.alloc_sbuf_tensor("ht", [128, D], BF16)
    # cols 0..31 = own rows; col 32 = donor score
    scores = nc.alloc_sbuf_tensor("scores", [128, RPP + 1], F32)
    sig = nc.alloc_sbuf_tensor("sigout", [128, RPP + 1], F32)

    h_sem = nc.alloc_semaphore("hld")
    head_sem = nc.alloc_semaphore("hd")
    msems = [nc.alloc_semaphore(f"m{i}") for i in range(len(MAIN_SIZES))]
    dsem = nc.alloc_semaphore("dn")
    tsems = [nc.alloc_semaphore(f"t{g}") for g in range(len(TAIL_SIZES))]
    tt_sem = nc.alloc_semaphore("tt")
    dtt_sem = nc.alloc_semaphore("dtt")
    sig_sem = nc.alloc_semaphore("sg")
    dsig_sem = nc.alloc_semaphore("dsg")
    outd_sem = nc.alloc_semaphore("outd")

    def erow(r):
        """bf16 slot of slice-row r (r >= HEAD_ROWS) in eall."""
        o = (r - HEAD_ROWS) * D
        return (o, o + D)

    main_r0 = np.cumsum([HEAD_ROWS] + MAIN_SIZES)  # first row of each load
    tail_r0 = np.cumsum([TAIL0] + TAIL_SIZES)

    with _HintedBlock(nc, f"blk{nc.next_id()}") as block:

        @block.gpsimd
        def _(g: bass.BassEngine):
            block.hint_locs[g] = g.mark_branch_hint_location()

            def main_load(i):
                r0, sz = int(main_r0[i]), MAIN_SIZES[i]
                g.dma_start(
                    out=eall.ap()[:, (r0 - HEAD_ROWS) * D:
                                  (r0 - HEAD_ROWS + sz) * D],
                    in_=ev3[:, r0:r0 + sz, :].rearrange("p r d -> p (r d)"),
                ).then_inc(msems[i], 16)

            main_load(0)
            main_load(1)
            # donor loads: slow partitions' rows 27..31 -> donor partitions
            for j, sp in enumerate([92, 93, 94, 95, 124, 125, 126, 127]):
                dp = 5 * j if j < 4 else 64 + 5 * (j - 4)
                g.dma_start(
                    out=extra.ap()[dp:dp + 5, :],
                    in_=ev3[sp, TAIL0:RPP, :],
                ).then_inc(dsem, 16)
            for i in range(2, len(MAIN_SIZES)):
                main_load(i)
            for gi, sz in enumerate(TAIL_SIZES):
                r0 = int(tail_r0[gi])
                a, b = erow(r0)[0], erow(r0 + sz - 1)[1]
                g.dma_start(
                    out=eall.ap()[0:92, a:b],
                    in_=ev3[0:92, r0:r0 + sz, :].rearrange("p r d -> p (r d)"),
                ).then_inc(tsems[gi], 16)
                g.dma_start(
                    out=eall.ap()[96:124, a:b],
                    in_=ev3[96:124, r0:r0 + sz, :].rearrange(
                        "p r d -> p (r d)"),
                ).then_inc(tsems[gi], 16)

        @block.vector
        def _(v: bass.BassEngine):
            block.hint_locs[v] = v.mark_branch_hint_location()
            v.wait_ge(h_sem, 16)
            v.tensor_copy(out=ht.ap(), in_=htf.ap())

            def ttr(in0, accum_col, in1):
                # out = (in0 * 1.0) * in1 ; accum_out = row-sum(out) in f32
                return v.scalar_tensor_tensor(
                    out=in0, in0=in0, scalar=1.0, in1=in1,
                    op0=mybir.AluOpType.mult, op1=mybir.AluOpType.mult,
                    accum_out=scores.ap()[:, accum_col:accum_col + 1],
                )

            # head rows in f32
            v.wait_ge(head_sem, 16)
            for r in range(HEAD_ROWS):
                ttr(eallf.ap()[:, r * D:(r + 1) * D], r,
                    htf.ap()).then_inc(tt_sem, 1)
            # main rows (bf16)
            for i, sz in enumerate(MAIN_SIZES):
                r0 = int(main_r0[i])
                v.wait_ge(msems[i], 16)
                for r in range(r0, r0 + sz):
                    a, b = erow(r)
                    ttr(eall.ap()[:, a:b], r, ht.ap()).then_inc(tt_sem, 1)
            # donor rows
            v.wait_ge(dsem, 128)
            ttr(extra.ap(), RPP, ht.ap()).then_inc(dtt_sem, 1)
            # tail rows (full-width TTR; slow partitions hold garbage there,
            # which is never stored)
            for gi, sz in enumerate(TAIL_SIZES):
                r0 = int(tail_r0[gi])
                v.wait_ge(tsems[gi], 32)
                for r in range(r0, r0 + sz):
                    a, b = erow(r)
                    ttr(eall.ap()[:, a:b], r, ht.ap()).then_inc(tt_sem, 1)

        @block.scalar
        def _(s: bass.BassEngine):
            block.hint_locs[s] = s.mark_branch_hint_location()
            # warm the sigmoid funcset off the critical tail
            cz = nc.const_aps.scalar_like(0.0, sig.ap()[:, 0:1])
            s.activation(out=sig.ap()[:, 0:1], in_=cz,
                         func=mybir.ActivationFunctionType.Sigmoid)
            s.wait_ge(dtt_sem, 1)
            s.activation(
                out=sig.ap()[:, RPP:RPP + 1], in_=scores.ap()[:, RPP:RPP + 1],
                func=mybir.ActivationFunctionType.Sigmoid,
            ).then_inc(dsig_sem, 1)
            s.wait_ge(tt_sem, TAIL0)
            s.activation(
                out=sig.ap()[:, :TAIL0], in_=scores.ap()[:, :TAIL0],
                func=mybir.ActivationFunctionType.Sigmoid,
            ).then_inc(sig_sem, 1)
            s.wait_ge(tt_sem, RPP)
            s.activation(
                out=sig.ap()[:, TAIL0:RPP], in_=scores.ap()[:, TAIL0:RPP],
                func=mybir.ActivationFunctionType.Sigmoid,
            ).then_inc(sig_sem, 2)

        @block.sync
        def _(sy: bass.BassEngine):
            block.hint_locs[sy] = sy.mark_branch_hint_location()
            sy.dma_start(
                out=htf.ap(),
                in_=h_dram.ap().unsqueeze(0).broadcast_to((128, D))
            ).then_inc(h_sem, 16)
            sy.dma_start(
                out=eallf.ap(),
                in_=ev3[:, 0:HEAD_ROWS, :].rearrange("p r d -> p (r d)"),
            ).then_inc(head_sem, 16)
            # donor scatter-stores (mid-kernel, off the critical path)
            sy.wait_ge(dsig_sem, 1)
            for j, sp in enumerate([92, 93, 94, 95, 124, 125, 126, 127]):
                dp = 5 * j if j < 4 else 64 + 5 * (j - 4)
                sy.dma_start(
                    out=o_rear[sp, TAIL0:RPP].unsqueeze(1),
                    in_=sig.ap()[dp:dp + 5, RPP:RPP + 1],
                ).then_inc(outd_sem, 16)
            sy.wait_ge(sig_sem, 1)
            sy.dma_start(out=o_rear[:, :TAIL0],
                         in_=sig.ap()[:, :TAIL0]).then_inc(outd_sem, 16)
            sy.wait_ge(sig_sem, 2)
            sy.dma_start(out=o_rear[0:92, TAIL0:RPP],
                         in_=sig.ap()[0:92, TAIL0:RPP]).then_inc(outd_sem, 16)
            sy.dma_start(
                out=o_rear[96:124, TAIL0:RPP],
                in_=sig.ap()[96:124, TAIL0:RPP]).then_inc(outd_sem, 16)
            sy.wait_ge(outd_sem, (8 + 3) * 16)

    nc.compile()
    return nc


def make_in_maps(hidden, encoder_outputs):
    hidden = np.ascontiguousarray(np.asarray(hidden, dtype=np.float32))
    encoder_outputs = np.asarray(encoder_outputs, dtype=np.float32)
    return [
        {"hidden": hidden,
         "encoder_outputs": np.ascontiguousarray(
             encoder_outputs[i * ROWS:(i + 1) * ROWS])}
        for i in range(N_CORES)
    ]


_NC_CACHE = None


def _get_nc():
    global _NC_CACHE
    if _NC_CACHE is None:
        _NC_CACHE = build()
    return _NC_CACHE


def _make_in_maps(hidden, encoder_outputs):
    return make_in_maps(hidden, encoder_outputs)


def kernel(hidden, encoder_outputs):
    nc = _get_nc()
    in_maps = make_in_maps(hidden, encoder_outputs)
    res = run_bass_kernel_spmd(nc, in_maps, core_ids=list(range(N_CORES)))
    out = np.concatenate(
        [np.asarray(res.results[i]["out"]).reshape(-1) for i in range(N_CORES)])
    return out[None, None, :].astype(np.float32)


# revision 13
# speedup vs baseline: 1.0903x; 1.0903x over previous
"""Trainium2 8-core Bass kernel: out = sigmoid(encoder_outputs @ hidden),
encoder_outputs [32768, 1024] f32, hidden [1024] f32 -> [1, 1, 32768] f32.

Sharding: encoder_outputs splits along seq_len into 8 slices of [4096, 1024]
(one per NeuronCore); hidden is replicated; each core produces its 4096
sigmoid scores and the host concatenates. No collectives needed.

Per-core kernel (raw bacc, hand-placed semaphores; no Tile machinery):
  - partition p owns rows [32p, 32p+32) of the slice; scores map to the
    output vector with per-partition contiguous stores
  - rows 0-1 load f32 via HWDGE (SP) before the SWDGE ring finishes
    initializing; rows 2-26 stream as full-width SWDGE cast-DMAs
    (f32 DRAM -> bf16 SBUF)
  - SDMA engine 15 (partitions 92-95/124-127 by the port swizzle) is the
    known-slow engine under SWDGE descriptor-ring contention, so those 8
    partitions only receive rows 0-26: their rows 27-31 are instead loaded
    into 40 "donor" partitions (0-19, 64-83) via two 3D-AP cast-DMAs,
    reduced there, and scatter-stored straight to the right DRAM words
    mid-kernel. Rows 27-31 for the remaining 120 partitions stream as
    partition-subset loads [0:92]/[96:124] that skip engine 15 entirely.
  - each row is multiplied and reduced in ONE DVE op
    (tensor_tensor_reduce: out = E_row * h, accum_out = row-sum in f32),
    so ScalarE only runs sigmoids and the post-last-byte chain is just
    TTR -> sigmoid -> store
  - a warm Sigmoid on the const-zero AP pulls the single ACT funcset load
    off the tail; branch hints arm each engine's end-block branch
Memory-bound at the per-engine SDMA line rate (~25.7 GB/s f32-read per
engine with cast); bf16 multiply keeps rel err ~5e-3 (gate 2e-2).
"""
import numpy as np
from concourse.bass_utils import run_bass_kernel_spmd


import concourse.bass as bass
from concourse import bacc, mybir


class _HintedBlock(bass.BassBlock):
    """no_gpsimd_drain block whose end-bb branches carry prefetch hints."""

    def __init__(self, bass_, name):
        super().__init__(bass_, name, no_gpsimd_drain=True)
        self.hint_locs = {}

    def __exit__(self, exc_type, exc_val, exc_tb):
        if exc_type is not None:
            return
        for engine, last_body in self.last_body.items():
            with self.bass.body(last_body, parent=self.bass.cur_bb,
                                allow_existing_parent=True):
                br = engine.br(self.end_bb)
                loc = self.hint_locs.get(engine)
                if loc is not None:
                    br.branch_hint(loc)
        self.bass.switch_bb(self.end_bb)
        gpsimd_type = self.bass.gpsimd.engine
        for eng_type, eng in self.bass.engines.items():
            if eng_type == gpsimd_type:
                continue
            d = mybir.InstDrain(
                name=self.bass.get_next_instruction_name(),
                ins=[], outs=[], bass_is_fusable=False)
            d.engine = eng_type
            eng.add_instruction(d)
        self.bass.all_engine_barrier(sem_only=True)

N_CORES = 8
SEQ = 32768
D = 1024
ROWS = SEQ // N_CORES          # 4096
RPP = ROWS // 128              # 32
F32 = mybir.dt.float32
BF16 = mybir.dt.bfloat16

HEAD_ROWS = 2                  # rows 0-1, f32 via HWDGE at kernel start
MAIN_SIZES = [4, 4, 4, 4, 4, 5]  # rows 2..26, full 128-partition SWDGE
TAIL_SIZES = [2, 1, 1, 1]        # rows 27..31, partitions [0:92]+[96:124]
# slow-engine partitions (SDMA engine 15): 92-95 and 124-127.
# their rows 27..31 are loaded by donor partitions [0:20] and [64:84].
N_MAIN_ROWS = sum(MAIN_SIZES)    # 25
N_TAIL_ROWS = sum(TAIL_SIZES)    # 5
TAIL0 = HEAD_ROWS + N_MAIN_ROWS  # 27
assert TAIL0 + N_TAIL_ROWS == RPP


def build():
    nc = bacc.Bacc("TRN2", target_bir_lowering=False, debug=False,
                   num_devices=N_CORES)
    h_dram = nc.dram_tensor("hidden", [D], F32, kind="ExternalInput")
    e_dram = nc.dram_tensor("encoder_outputs", [ROWS, D], F32,
                            kind="ExternalInput")
    o_dram = nc.dram_tensor("out", [ROWS], F32, kind="ExternalOutput")
    ev3 = e_dram.ap().rearrange("(p r) d -> p r d", p=128)   # [128, 32, D]
    o_rear = o_dram.ap().rearrange("(p r) -> p r", p=128)    # [128, 32]

    eallf = nc.alloc_sbuf_tensor("eallf", [128, HEAD_ROWS * D], F32)
    eall = nc.alloc_sbuf_tensor("eall", [128, (RPP - HEAD_ROWS) * D], BF16)
    extra = nc.alloc_sbuf_tensor("extra", [128, D], BF16)  # donor rows
    htf = nc.alloc_sbuf_tensor("htf", [128, D], F32)
    ht = nc.alloc_sbuf_tensor("ht", [128, D], BF16)
    prodf = nc.alloc_sbuf_tensor("prodf", [128, HEAD_ROWS * D], BF16)
    prods = [nc.alloc_sbuf_tensor(f"prod{i}", [128, sz * D], BF16)
             for i, sz in enumerate(MAIN_SIZES)]
    prodt = nc.alloc_sbuf_tensor("prodt", [128, TAIL_SIZES[0] * D], BF16)
    # cols 0..31 = own rows; col 32 = donor score
    scores = nc.alloc_sbuf_tensor("scores", [128, RPP + 1], F32)
    sig = nc.alloc_sbuf_tensor("sigout", [128, RPP + 1], F32)

    h_sem = nc.alloc_semaphore("hld")
    head_sem = nc.alloc_semaphore("hd")
    msems = [nc.alloc_semaphore(f"m{i}") for i in range(len(MAIN_SIZES))]
    dsem = nc.alloc_semaphore("dn")
    tsems = [nc.alloc_semaphore(f"t{g}") for g in range(len(TAIL_SIZES))]
    tt_sem = nc.alloc_semaphore("tt")    # DVE tensor_tensor completions
    row_sem = nc.alloc_semaphore("row")   # rows 0..26 score completions
    trow_sem = nc.alloc_semaphore("trow")  # rows 27..31 score completions
    dtt_sem = nc.alloc_semaphore("dtt")
    sig_sem = nc.alloc_semaphore("sg")
    dsig_sem = nc.alloc_semaphore("dsg")
    outd_sem = nc.alloc_semaphore("outd")

    def erow(r):
        """bf16 slot of slice-row r (r >= HEAD_ROWS) in eall."""
        o = (r - HEAD_ROWS) * D
        return (o, o + D)

    main_r0 = np.cumsum([HEAD_ROWS] + MAIN_SIZES)  # first row of each load
    tail_r0 = np.cumsum([TAIL0] + TAIL_SIZES)

    with _HintedBlock(nc, f"blk{nc.next_id()}") as block:

        @block.gpsimd
        def _(g: bass.BassEngine):
            block.hint_locs[g] = g.mark_branch_hint_location()

            def main_load(i):
                r0, sz = int(main_r0[i]), MAIN_SIZES[i]
                g.dma_start(
                    out=eall.ap()[:, (r0 - HEAD_ROWS) * D:
                                  (r0 - HEAD_ROWS + sz) * D],
                    in_=ev3[:, r0:r0 + sz, :].rearrange("p r d -> p (r d)"),
                ).then_inc(msems[i], 16)

            main_load(0)
            main_load(1)
            # donor loads: slow partitions' rows 27..31 -> donor partitions
            for j, sp in enumerate([92, 93, 94, 95, 124, 125, 126, 127]):
                dp = 5 * j if j < 4 else 64 + 5 * (j - 4)
                g.dma_start(
                    out=extra.ap()[dp:dp + 5, :],
                    in_=ev3[sp, TAIL0:RPP, :],
                ).then_inc(dsem, 16)
            for i in range(2, len(MAIN_SIZES)):
                main_load(i)
            for gi, sz in enumerate(TAIL_SIZES):
                r0 = int(tail_r0[gi])
                a, b = erow(r0)[0], erow(r0 + sz - 1)[1]
                g.dma_start(
                    out=eall.ap()[0:92, a:b],
                    in_=ev3[0:92, r0:r0 + sz, :].rearrange("p r d -> p (r d)"),
                ).then_inc(tsems[gi], 16)
                g.dma_start(
                    out=eall.ap()[96:124, a:b],
                    in_=ev3[96:124, r0:r0 + sz, :].rearrange(
                        "p r d -> p (r d)"),
                ).then_inc(tsems[gi], 16)

        @block.vector
        def _(v: bass.BassEngine):
            block.hint_locs[v] = v.mark_branch_hint_location()
            v.wait_ge(h_sem, 16)
            v.tensor_copy(out=ht.ap(), in_=htf.ap())

            def stt(in0, accum_col, in1):
                # fused: out = in0 * in1 ; accum_out = row-sum (f32)
                return v.scalar_tensor_tensor(
                    out=in0, in0=in0, scalar=1.0, in1=in1,
                    op0=mybir.AluOpType.mult, op1=mybir.AluOpType.mult,
                    accum_out=scores.ap()[:, accum_col:accum_col + 1],
                )

            def tt_batch(dst, src, sz, hvec):
                return v.tensor_tensor(
                    out=dst.rearrange("p (r d) -> p r d", r=sz),
                    in0=src.rearrange("p (r d) -> p r d", r=sz),
                    in1=hvec.unsqueeze(1).broadcast_to((128, sz, D)),
                    op=mybir.AluOpType.mult,
                )

            def reduce1(src, col):
                return v.tensor_reduce(
                    out=scores.ap()[:, col:col + 1],
                    in_=src.rearrange("p (r d) -> p r d", r=1),
                    axis=mybir.AxisListType.X, op=mybir.AluOpType.add,
                )

            # head rows in f32 -> bf16 products
            v.wait_ge(head_sem, 16)
            tt_batch(prodf.ap(), eallf.ap(), HEAD_ROWS,
                     htf.ap()).then_inc(tt_sem, 1)
            # main loads: batched TT (2x packed bf16); DVE reduces first row,
            # ACT accumulates the rest
            for i, sz in enumerate(MAIN_SIZES):
                r0 = int(main_r0[i])
                v.wait_ge(msems[i], 16)
                tt_batch(prods[i].ap(), eall.ap()[:, (r0 - HEAD_ROWS) * D:
                                                  (r0 - HEAD_ROWS + sz) * D],
                         sz, ht.ap()).then_inc(tt_sem, 1)
                reduce1(prods[i].ap()[:, 0:D], r0).then_inc(row_sem, 1)
                if i == 2:
                    # donor rows: fused multiply+reduce, mid-kernel
                    v.wait_ge(dsem, 128)
                    stt(extra.ap(), RPP, ht.ap()).then_inc(dtt_sem, 1)
            # tail group 0 (2 rows): TT + ACT accums
            r0 = int(tail_r0[0])
            v.wait_ge(tsems[0], 32)
            tt_batch(prodt.ap(), eall.ap()[:, (r0 - HEAD_ROWS) * D:
                                           (r0 - HEAD_ROWS + TAIL_SIZES[0]) * D],
                     TAIL_SIZES[0], ht.ap()).then_inc(tt_sem, 1)
            # tail single rows: fused STT straight into scores
            for gi in range(1, len(TAIL_SIZES)):
                r = int(tail_r0[gi])
                a, b = erow(r)
                v.wait_ge(tsems[gi], 32)
                stt(eall.ap()[:, a:b], r, ht.ap()).then_inc(trow_sem, 1)

        @block.scalar
        def _(s: bass.BassEngine):
            block.hint_locs[s] = s.mark_branch_hint_location()
            # warm the sigmoid funcset off the critical tail
            cz = nc.const_aps.scalar_like(0.0, sig.ap()[:, 0:1])
            s.activation(out=sig.ap()[:, 0:1], in_=cz,
                         func=mybir.ActivationFunctionType.Sigmoid)

            def accum(src, col):
                return s.activation(
                    out=src, in_=src,
                    func=mybir.ActivationFunctionType.Copy,
                    accum_out=scores.ap()[:, col:col + 1],
                ).then_inc(row_sem, 1)

            # head rows
            s.wait_ge(tt_sem, 1)
            for r in range(HEAD_ROWS):
                accum(prodf.ap()[:, r * D:(r + 1) * D], r)
            # main rows beyond each load's first
            for i, sz in enumerate(MAIN_SIZES):
                r0 = int(main_r0[i])
                s.wait_ge(tt_sem, 2 + i)
                for j in range(1, sz):
                    accum(prods[i].ap()[:, j * D:(j + 1) * D], r0 + j)
                if i == 3:
                    s.wait_ge(dtt_sem, 1)
                    s.activation(
                        out=sig.ap()[:, RPP:RPP + 1],
                        in_=scores.ap()[:, RPP:RPP + 1],
                        func=mybir.ActivationFunctionType.Sigmoid,
                    ).then_inc(dsig_sem, 1)
            # tail group 0 rows
            s.wait_ge(tt_sem, 2 + len(MAIN_SIZES))
            for j in range(TAIL_SIZES[0]):
                s.activation(
                    out=prodt.ap()[:, j * D:(j + 1) * D],
                    in_=prodt.ap()[:, j * D:(j + 1) * D],
                    func=mybir.ActivationFunctionType.Copy,
                    accum_out=scores.ap()[:, int(tail_r0[0]) + j:
                                          int(tail_r0[0]) + j + 1],
                ).then_inc(trow_sem, 1)
            s.wait_ge(row_sem, TAIL0)
            s.activation(
                out=sig.ap()[:, :TAIL0], in_=scores.ap()[:, :TAIL0],
                func=mybir.ActivationFunctionType.Sigmoid,
            ).then_inc(sig_sem, 1)
            s.wait_ge(trow_sem, RPP - TAIL0)
            s.activation(
                out=sig.ap()[:, TAIL0:RPP], in_=scores.ap()[:, TAIL0:RPP],
                func=mybir.ActivationFunctionType.Sigmoid,
            ).then_inc(sig_sem, 2)

        @block.sync
        def _(sy: bass.BassEngine):
            block.hint_locs[sy] = sy.mark_branch_hint_location()
            sy.dma_start(
                out=htf.ap(),
                in_=h_dram.ap().unsqueeze(0).broadcast_to((128, D))
            ).then_inc(h_sem, 16)
            sy.dma_start(
                out=eallf.ap(),
                in_=ev3[:, 0:HEAD_ROWS, :].rearrange("p r d -> p (r d)"),
            ).then_inc(head_sem, 16)
            # donor scatter-stores (mid-kernel, off the critical path)
            sy.wait_ge(dsig_sem, 1)
            for j, sp in enumerate([92, 93, 94, 95, 124, 125, 126, 127]):
                dp = 5 * j if j < 4 else 64 + 5 * (j - 4)
                sy.dma_start(
                    out=o_rear[sp, TAIL0:RPP].unsqueeze(1),
                    in_=sig.ap()[dp:dp + 5, RPP:RPP + 1],
                ).then_inc(outd_sem, 16)
            sy.wait_ge(sig_sem, 1)
            sy.dma_start(out=o_rear[:, :TAIL0],
                         in_=sig.ap()[:, :TAIL0]).then_inc(outd_sem, 16)
            sy.wait_ge(sig_sem, 2)
            sy.dma_start(out=o_rear[0:92, TAIL0:RPP],
                         in_=sig.ap()[0:92, TAIL0:RPP]).then_inc(outd_sem, 16)
            sy.dma_start(
                out=o_rear[96:124, TAIL0:RPP],
                in_=sig.ap()[96:124, TAIL0:RPP]).then_inc(outd_sem, 16)
            sy.wait_ge(outd_sem, (8 + 3) * 16)

    nc.compile()
    return nc


def make_in_maps(hidden, encoder_outputs):
    hidden = np.ascontiguousarray(np.asarray(hidden, dtype=np.float32))
    encoder_outputs = np.asarray(encoder_outputs, dtype=np.float32)
    return [
        {"hidden": hidden,
         "encoder_outputs": np.ascontiguousarray(
             encoder_outputs[i * ROWS:(i + 1) * ROWS])}
        for i in range(N_CORES)
    ]


_NC_CACHE = None


def _get_nc():
    global _NC_CACHE
    if _NC_CACHE is None:
        _NC_CACHE = build()
    return _NC_CACHE


def _make_in_maps(hidden, encoder_outputs):
    return make_in_maps(hidden, encoder_outputs)


def kernel(hidden, encoder_outputs):
    nc = _get_nc()
    in_maps = make_in_maps(hidden, encoder_outputs)
    res = run_bass_kernel_spmd(nc, in_maps, core_ids=list(range(N_CORES)))
    out = np.concatenate(
        [np.asarray(res.results[i]["out"]).reshape(-1) for i in range(N_CORES)])
    return out[None, None, :].astype(np.float32)


# revision 14
# speedup vs baseline: 1.2175x; 1.1167x over previous
"""Trainium2 8-core Bass kernel: out = sigmoid(encoder_outputs @ hidden),
encoder_outputs [32768, 1024] f32, hidden [1024] f32 -> [1, 1, 32768] f32.

Sharding: encoder_outputs splits along seq_len into 8 slices of [4096, 1024]
(one per NeuronCore); hidden is replicated; each core produces its 4096
sigmoid scores and the host concatenates. No collectives needed.

Per-core kernel (raw bacc, hand-placed semaphores; no Tile machinery):
  - partition p owns rows [32p, 32p+32) of the slice, so scores map to the
    output vector with per-partition contiguous stores
  - hidden + rows 0-1 load f32 via HWDGE (SP) before the SWDGE ring
    finishes initializing, engaging HBM early; the head rows are
    multiplied in f32
  - rows 2..31 stream as full-width (128-partition) SWDGE cast-DMAs
    (f32 DRAM -> bf16 SBUF) in 7 loads [6,6,6,6,3,2,1]: full-width ops
    distribute descriptors uniformly across all 16 SDMA engines
    (partition-subset ops get chunked onto arbitrary engines and cannot
    steer work away from the slow engine 15, measured), and fewer/bigger
    loads cut descriptor-ring traffic; the tapered tail keeps the final
    row's chain short
  - per load, one bf16 VectorEngine tensor_tensor (2x packed) multiplies
    its rows against hidden; the first row reduces on DVE (tensor_reduce),
    the rest accumulate on ScalarE (activation Copy + accum), all into
    f32 scores
  - the LAST row uses a fused scalar_tensor_tensor (out = row * h,
    accum_out = row-sum in f32) so the post-last-byte chain is just
    STT -> sigmoid -> store
  - a warm Sigmoid on the const-zero AP pulls the single ACT funcset load
    off the tail; branch hints arm each engine's end-block branch
Memory-bound at the per-engine SDMA cast line rate; engine 15 is ~20%
slower (known silicon quirk) and gates the stream. bf16 multiply keeps
rel err ~5e-3 (gate 2e-2).
"""
import numpy as np
from concourse.bass_utils import run_bass_kernel_spmd


import concourse.bass as bass
from concourse import bacc, mybir


class _HintedBlock(bass.BassBlock):
    """no_gpsimd_drain block whose end-bb branches carry prefetch hints."""

    def __init__(self, bass_, name):
        super().__init__(bass_, name, no_gpsimd_drain=True)
        self.hint_locs = {}

    def __exit__(self, exc_type, exc_val, exc_tb):
        if exc_type is not None:
            return
        for engine, last_body in self.last_body.items():
            with self.bass.body(last_body, parent=self.bass.cur_bb,
                                allow_existing_parent=True):
                br = engine.br(self.end_bb)
                loc = self.hint_locs.get(engine)
                if loc is not None:
                    br.branch_hint(loc)
        self.bass.switch_bb(self.end_bb)
        gpsimd_type = self.bass.gpsimd.engine
        for eng_type, eng in self.bass.engines.items():
            if eng_type == gpsimd_type:
                continue
            d = mybir.InstDrain(
                name=self.bass.get_next_instruction_name(),
                ins=[], outs=[], bass_is_fusable=False)
            d.engine = eng_type
            eng.add_instruction(d)
        self.bass.all_engine_barrier(sem_only=True)

N_CORES = 8
SEQ = 32768
D = 1024
ROWS = SEQ // N_CORES          # 4096
RPP = ROWS // 128              # 32
F32 = mybir.dt.float32
BF16 = mybir.dt.bfloat16

HEAD_ROWS = 2                    # rows 0-1, f32 via HWDGE
MAIN_SIZES = [6, 6, 6, 6, 3]     # rows 2..28: TT + DVE/ACT reduce split
TAIL2 = 2                        # rows 29-30: TT + ACT accums
# row 31: fused scalar_tensor_tensor straight into scores
SIG1 = HEAD_ROWS + sum(MAIN_SIZES)   # 29: first sigmoid covers cols < 29
assert SIG1 + TAIL2 + 1 == RPP


def build():
    nc = bacc.Bacc("TRN2", target_bir_lowering=False, debug=False,
                   num_devices=N_CORES)
    h_dram = nc.dram_tensor("hidden", [D], F32, kind="ExternalInput")
    e_dram = nc.dram_tensor("encoder_outputs", [ROWS, D], F32,
                            kind="ExternalInput")
    o_dram = nc.dram_tensor("out", [ROWS], F32, kind="ExternalOutput")
    ev3 = e_dram.ap().rearrange("(p r) d -> p r d", p=128)   # [128, 32, D]
    o_rear = o_dram.ap().rearrange("(p r) -> p r", p=128)    # [128, 32]

    eallf = nc.alloc_sbuf_tensor("eallf", [128, HEAD_ROWS * D], F32)
    eall = nc.alloc_sbuf_tensor("eall", [128, (RPP - HEAD_ROWS) * D], BF16)
    htf = nc.alloc_sbuf_tensor("htf", [128, D], F32)
    ht = nc.alloc_sbuf_tensor("ht", [128, D], BF16)
    prodf = nc.alloc_sbuf_tensor("prodf", [128, HEAD_ROWS * D], BF16)
    prods = [nc.alloc_sbuf_tensor(f"prod{i}", [128, sz * D], BF16)
             for i, sz in enumerate(MAIN_SIZES)]
    prodt = nc.alloc_sbuf_tensor("prodt", [128, TAIL2 * D], BF16)
    scores = nc.alloc_sbuf_tensor("scores", [128, RPP], F32)
    sig = nc.alloc_sbuf_tensor("sigout", [128, RPP], F32)

    h_sem = nc.alloc_semaphore("hld")
    head_sem = nc.alloc_semaphore("hd")
    n_loads = len(MAIN_SIZES) + 2          # main + 2-row tail + 1-row tail
    lsems = [nc.alloc_semaphore(f"l{i}") for i in range(n_loads)]
    tt_sem = nc.alloc_semaphore("tt")      # DVE tensor_tensor completions
    row_sem = nc.alloc_semaphore("row")    # rows 0..SIG1-1 completions
    trow_sem = nc.alloc_semaphore("trow")  # rows SIG1..31 completions
    sig_sem = nc.alloc_semaphore("sg")
    outd_sem = nc.alloc_semaphore("outd")

    main_r0 = np.cumsum([HEAD_ROWS] + MAIN_SIZES)  # first row of each load

    def eslot(r0, r1):
        return eall.ap()[:, (r0 - HEAD_ROWS) * D:(r1 - HEAD_ROWS) * D]

    with _HintedBlock(nc, f"blk{nc.next_id()}") as block:

        @block.gpsimd
        def _(g: bass.BassEngine):
            block.hint_locs[g] = g.mark_branch_hint_location()
            spans = [(int(main_r0[i]), int(main_r0[i]) + sz)
                     for i, sz in enumerate(MAIN_SIZES)]
            spans += [(SIG1, SIG1 + TAIL2), (SIG1 + TAIL2, RPP)]
            for i, (r0, r1) in enumerate(spans):
                g.dma_start(
                    out=eslot(r0, r1),
                    in_=ev3[:, r0:r1, :].rearrange("p r d -> p (r d)"),
                ).then_inc(lsems[i], 16)

        @block.vector
        def _(v: bass.BassEngine):
            block.hint_locs[v] = v.mark_branch_hint_location()
            v.wait_ge(h_sem, 16)
            v.tensor_copy(out=ht.ap(), in_=htf.ap())

            def tt_batch(dst, src, sz, hvec):
                return v.tensor_tensor(
                    out=dst.rearrange("p (r d) -> p r d", r=sz),
                    in0=src.rearrange("p (r d) -> p r d", r=sz),
                    in1=hvec.unsqueeze(1).broadcast_to((128, sz, D)),
                    op=mybir.AluOpType.mult,
                )

            # head rows in f32 -> bf16 products
            v.wait_ge(head_sem, 16)
            tt_batch(prodf.ap(), eallf.ap(), HEAD_ROWS,
                     htf.ap()).then_inc(tt_sem, 1)
            # main loads: batched TT; DVE reduces the first row, ACT the rest
            for i, sz in enumerate(MAIN_SIZES):
                r0 = int(main_r0[i])
                v.wait_ge(lsems[i], 16)
                tt_batch(prods[i].ap(), eslot(r0, r0 + sz), sz,
                         ht.ap()).then_inc(tt_sem, 1)
                v.tensor_reduce(
                    out=scores.ap()[:, r0:r0 + 1],
                    in_=prods[i].ap()[:, 0:D].rearrange(
                        "p (r d) -> p r d", r=1),
                    axis=mybir.AxisListType.X, op=mybir.AluOpType.add,
                ).then_inc(row_sem, 1)
            # rows 29-30: TT, ACT accumulates both
            v.wait_ge(lsems[len(MAIN_SIZES)], 16)
            tt_batch(prodt.ap(), eslot(SIG1, SIG1 + TAIL2), TAIL2,
                     ht.ap()).then_inc(tt_sem, 1)
            # row 31: fused multiply+reduce straight into scores
            v.wait_ge(lsems[len(MAIN_SIZES) + 1], 16)
            v.scalar_tensor_tensor(
                out=eslot(RPP - 1, RPP), in0=eslot(RPP - 1, RPP),
                scalar=1.0, in1=ht.ap(),
                op0=mybir.AluOpType.mult, op1=mybir.AluOpType.mult,
                accum_out=scores.ap()[:, RPP - 1:RPP],
            ).then_inc(trow_sem, 1)

        @block.scalar
        def _(s: bass.BassEngine):
            block.hint_locs[s] = s.mark_branch_hint_location()
            # warm the sigmoid funcset off the critical tail
            cz = nc.const_aps.scalar_like(0.0, sig.ap()[:, 0:1])
            s.activation(out=sig.ap()[:, 0:1], in_=cz,
                         func=mybir.ActivationFunctionType.Sigmoid)

            def accum(src, col, sem):
                return s.activation(
                    out=src, in_=src,
                    func=mybir.ActivationFunctionType.Copy,
                    accum_out=scores.ap()[:, col:col + 1],
                ).then_inc(sem, 1)

            s.wait_ge(tt_sem, 1)
            for r in range(HEAD_ROWS):
                accum(prodf.ap()[:, r * D:(r + 1) * D], r, row_sem)
            for i, sz in enumerate(MAIN_SIZES):
                r0 = int(main_r0[i])
                s.wait_ge(tt_sem, 2 + i)
                for j in range(1, sz):
                    accum(prods[i].ap()[:, j * D:(j + 1) * D], r0 + j,
                          row_sem)
            s.wait_ge(tt_sem, 2 + len(MAIN_SIZES))
            for j in range(TAIL2):
                accum(prodt.ap()[:, j * D:(j + 1) * D], SIG1 + j, trow_sem)
            s.wait_ge(row_sem, SIG1)
            s.activation(
                out=sig.ap()[:, :SIG1], in_=scores.ap()[:, :SIG1],
                func=mybir.ActivationFunctionType.Sigmoid,
            ).then_inc(sig_sem, 1)
            s.wait_ge(trow_sem, RPP - SIG1)
            s.activation(
                out=sig.ap()[:, SIG1:], in_=scores.ap()[:, SIG1:],
                func=mybir.ActivationFunctionType.Sigmoid,
            ).then_inc(sig_sem, 2)

        @block.sync
        def _(sy: bass.BassEngine):
            block.hint_locs[sy] = sy.mark_branch_hint_location()
            sy.dma_start(
                out=htf.ap(),
                in_=h_dram.ap().unsqueeze(0).broadcast_to((128, D))
            ).then_inc(h_sem, 16)
            sy.dma_start(
                out=eallf.ap(),
                in_=ev3[:, 0:HEAD_ROWS, :].rearrange("p r d -> p (r d)"),
            ).then_inc(head_sem, 16)
            sy.wait_ge(sig_sem, 1)
            sy.dma_start(out=o_rear[:, :SIG1],
                         in_=sig.ap()[:, :SIG1]).then_inc(outd_sem, 16)
            sy.wait_ge(sig_sem, 2)
            sy.dma_start(out=o_rear[:, SIG1:],
                         in_=sig.ap()[:, SIG1:]).then_inc(outd_sem, 16)
            sy.wait_ge(outd_sem, 32)

    nc.compile()
    return nc


def make_in_maps(hidden, encoder_outputs):
    hidden = np.ascontiguousarray(np.asarray(hidden, dtype=np.float32))
    encoder_outputs = np.asarray(encoder_outputs, dtype=np.float32)
    return [
        {"hidden": hidden,
         "encoder_outputs": np.ascontiguousarray(
             encoder_outputs[i * ROWS:(i + 1) * ROWS])}
        for i in range(N_CORES)
    ]


_NC_CACHE = None


def _get_nc():
    global _NC_CACHE
    if _NC_CACHE is None:
        _NC_CACHE = build()
    return _NC_CACHE


def _make_in_maps(hidden, encoder_outputs):
    return make_in_maps(hidden, encoder_outputs)


def kernel(hidden, encoder_outputs):
    nc = _get_nc()
    in_maps = make_in_maps(hidden, encoder_outputs)
    res = run_bass_kernel_spmd(nc, in_maps, core_ids=list(range(N_CORES)))
    out = np.concatenate(
        [np.asarray(res.results[i]["out"]).reshape(-1) for i in range(N_CORES)])
    return out[None, None, :].astype(np.float32)
